# revision 28
# baseline (speedup 1.0000x reference)
"""Trainium2 Bass kernel for causal retention multi-head attention.

    kqv = x @ W1 + b1 ; k,q,v = split(kqv)
    out_h = tril_mask(q_h k_h^T) v_h        (per head, no softmax)
    out = concat_heads @ W2 + b2

Sharding: tensor-parallel over heads (2 heads per core). Retention is linear
attention, so inter-chunk contributions collapse into a running 64x64 state
per head:

    O_i = Q_i @ State_{<i} + tril(Q_i K_i^T) V_i
    State_{<i+1} = State_{<i} + K_i^T V_i

Only the block-diagonal 512x512 score tiles are materialized; everything
before the current chunk flows through the state. This cuts attention matmul
work ~5x and PSUM->SBUF score copies ~4x vs. materializing the full causal
staircase.

After attention, an AllToAll exchanges head-feature slices for sequence
slices so each core applies the full W2 to its rows. Group {0..5} launches
after chunk 5 (hidden by remaining compute); the small {6,7} group forms the
tail.
"""
import numpy as np

import concourse.bacc as bacc
import concourse.bass as bass
import concourse.mybir as mybir
import concourse.tile as tile
from concourse.bass_utils import run_bass_kernel_spmd

dt = mybir.dt
ts = bass.ts

T = 4096          # sequence length
D = 1024          # embed dim
NCORES = 8
FPC = D // NCORES  # feature columns per core = 128 (2 heads x 64)
DH = 64           # head dim
TC = 256          # t-chunk
NT = T // TC      # 16 chunks
SB = 128          # s-block within the diagonal chunk
NB = TC // SB     # s-blocks per chunk

# A2A exchange groups: (chunk list, shard width, out row base)
GROUPS = [(list(range(12)), 384, 0), ([12, 13, 14, 15], 128, 384)]


class EngineBalancer:
    """Greedy assignment of PSUM->SBUF ops to the less-loaded of DVE/ACT.

    Costs in ns from the TRN2 cost model: DVE 1.042 ns/col (0.521 with the
    2x packed-16-bit mode), ACT 0.833 ns/col, plus memory-access init.
    tensor_tensor ops exist only on DVE.
    """

    def __init__(self, nc):
        self.nc = nc
        self.busy = {"dve": 0.0, "act": 1300.0}  # ACT pays a one-time LUT load

    def _dve_cost(self, cols, psum, b16):
        return cols * (0.521 if b16 else 1.042) + (125.0 if psum else 60.0)

    def _act_cost(self, cols, psum):
        return cols * 0.833 + (143.0 if psum else 185.0)

    def _pick(self, dve_cost, act_cost):
        if self.busy["dve"] + dve_cost <= self.busy["act"] + act_cost:
            self.busy["dve"] += dve_cost
            return "dve"
        self.busy["act"] += act_cost
        return "act"

    def copy(self, out_ap, in_ap, cols, psum=True, b16=False):
        if self._pick(self._dve_cost(cols, psum, b16),
                      self._act_cost(cols, psum)) == "dve":
            self.nc.vector.tensor_copy(out_ap, in_ap)
        else:
            self.nc.scalar.copy(out_ap, in_ap)

    def mul_mask(self, out_ap, in_ap, mask_ap, cols):
        # tensor_tensor only exists on DVE
        self.busy["dve"] += self._dve_cost(cols, True, False)
        self.nc.vector.tensor_mul(out_ap, in_ap, mask_ap)

    def bias_add(self, out_ap, in_ap, bias_ap, cols):
        if self._pick(self._dve_cost(cols, True, False),
                      self._act_cost(cols, True)) == "dve":
            self.nc.vector.tensor_scalar_add(out_ap, in_ap, bias_ap)
        else:
            self.nc.scalar.activation(
                out_ap, in_ap, mybir.ActivationFunctionType.Identity,
                bias=bias_ap, scale=1.0)

    def tensor_add(self, out_ap, a_ap, b_ap, cols):
        self.busy["dve"] += self._dve_cost(cols, True, False)
        self.nc.vector.tensor_add(out_ap, a_ap, b_ap)


def build_program(reps: int = 1, xsplit: int = 8,
                  fake_cc: bool = False) -> bass.Bass:
    nc = bacc.Bacc("TRN2", target_bir_lowering=False, debug=False,
                   num_devices=NCORES)

    xT = nc.dram_tensor("xT", [D, T], dt.bfloat16, kind="ExternalInput")
    w1l = nc.dram_tensor("w1l", [D, 3 * FPC], dt.bfloat16, kind="ExternalInput")
    b1l = nc.dram_tensor("b1l", [3, FPC], dt.float32, kind="ExternalInput")
    w2 = nc.dram_tensor("w2", [D, D], dt.bfloat16, kind="ExternalInput")
    b2 = nc.dram_tensor("b2", [D], dt.float32, kind="ExternalInput")
    masktri = nc.dram_tensor("masktri", [SB, SB], dt.float32,
                             kind="ExternalInput")
    ident = nc.dram_tensor("ident", [SB, SB], dt.bfloat16, kind="ExternalInput")
    out = nc.dram_tensor("out", [T // NCORES, D], dt.float32,
                         kind="ExternalOutput")

    rg = [list(range(NCORES))]

    with tile.TileContext(nc) as tc:
        with (
            tc.tile_pool(name="res", bufs=1) as res,          # resident tensors
            tc.tile_pool(name="stage", bufs=3) as stage,      # vtmp / att staging
            tc.tile_pool(name="ksp", bufs=2) as ksp,          # K[s,d] per chunk
            tc.tile_pool(name="sp", bufs=4) as sp,            # masked score tiles
            tc.tile_pool(name="stp", bufs=2) as stp,          # state bf16
            tc.tile_pool(name="ost", bufs=4) as ostp,         # W2 output staging
            tc.tile_pool(name="rx", bufs=2) as rxp,          # post-A2A tiles
            tc.tile_pool(name="psA", bufs=2, space="PSUM") as psA,
            tc.tile_pool(name="psT", bufs=1, space="PSUM") as psT,
            tc.tile_pool(name="psS", bufs=3, space="PSUM") as psS,
            tc.tile_pool(name="psO", bufs=1, space="PSUM") as psO,
            tc.tile_pool(name="psStat", bufs=1, space="PSUM") as psStat,
            tc.tile_pool(name="dram", bufs=1, space="DRAM") as dram,
        ):
            eb = EngineBalancer(nc)
            # ---- resident loads -------------------------------------------
            w1sb = res.tile([128, D // 128, 3 * FPC], dt.bfloat16, tag="w1")
            nc.sync.dma_start(w1sb[:], w1l.ap().rearrange("(a p) c -> p a c", p=128))
            b1sb = res.tile([FPC, 3], dt.float32, tag="b1")
            nc.sync.dma_start(b1sb[:], b1l.ap().rearrange("m p -> p m"))
            identsb = res.tile([SB, SB], dt.bfloat16, tag="ident")
            nc.sync.dma_start(identsb[:], ident.ap())
            masksb = res.tile([SB, SB], dt.float32, tag="masktri")

            xsb = res.tile([128, D // 128, T], dt.bfloat16, tag="x", name="xsb")
            seg = T // xsplit
            for sg in range(xsplit):
                for kc in range(D // 128):
                    nc.sync.dma_start(
                        xsb[:, kc, sg * seg:(sg + 1) * seg],
                        xT[kc * 128:(kc + 1) * 128, sg * seg:(sg + 1) * seg])

            w2sb = res.tile([128, D // 128, D], dt.bfloat16, tag="w2")
            b2row = res.tile([1, D], dt.float32, tag="b2row")
            b2bc = res.tile([128, D], dt.float32, tag="b2bc")

            def deferred_loads_1():
                nc.sync.dma_start(masksb[:], masktri.ap())

            def deferred_loads_2():
                nc.sync.dma_start(w2sb[:],
                                  w2.ap().rearrange("(a p) c -> p a c", p=128))
                nc.sync.dma_start(b2row[:],
                                  b2.ap().rearrange("(o c) -> o c", o=1))
                nc.gpsimd.partition_broadcast(b2bc[:], b2row[:])

            kTsb = res.tile([FPC, T], dt.bfloat16, tag="kT")
            qTsb = res.tile([FPC, T], dt.bfloat16, tag="qT")
            vsb = res.tile([SB, T // SB, FPC], dt.bfloat16, tag="v")

            # A2A staging
            a2a_in, a2a_out = [], []
            for g, (chunks, W, base) in enumerate(GROUPS):
                a2a_in.append(dram.tile([NCORES, FPC, W], dt.bfloat16,
                                        tag=f"ain{g}", name=f"a2a_in{g}"))
                a2a_out.append(dram.tile([NCORES, FPC, W], dt.bfloat16,
                                         tag=f"aout{g}", name=f"a2a_out{g}"))
            chunk_group = {}
            for g, (chunks, W, base) in enumerate(GROUPS):
                for pos, i in enumerate(chunks):
                    chunk_group[i] = (g, pos)

            def a2a_launch(g: int):
                if fake_cc:
                    nc.gpsimd.dma_start(a2a_out[g][:], a2a_in[g][:])
                    return
                nc.gpsimd.collective_compute(
                    "AllToAll", mybir.AluOpType.bypass, replica_groups=rg,
                    ins=[a2a_in[g].opt()], outs=[a2a_out[g].opt()],
                )

            def rx_load(g: int):
                """One rearranged DMA per group. Queue choice matters: a DMA
                that waits on its collective parks in the issuing queue and
                yields to ALL later-issued ready work there, so rx0 sits
                alone on SP (fires the moment collective 0 lands) and rx1
                follows launch(1) in the Pool queue (in-order, no inversion).
                """
                W = GROUPS[g][1]
                r = rxp.tile([FPC, NCORES, W], dt.bfloat16, tag=f"rx{g}",
                             name=f"rx{g}")
                eng = nc.sync if g == 0 else nc.gpsimd
                eng.dma_start(r[:], a2a_out[g].rearrange("f p w -> p f w"))
                return r

            def att_store(i, att):
                g, pos = chunk_group[i]
                W = GROUPS[g][1]
                a = TC * pos
                while a < TC * pos + TC:
                    s = a // W
                    e = min((s + 1) * W, TC * pos + TC)
                    nc.sync.dma_start(
                        a2a_in[g][s][:, a - s * W: e - s * W],
                        att[:, a - TC * pos: e - TC * pos])
                    a = e

            def w2_group(g: int, rxt):
                chunks, W, base = GROUPS[g]
                for tt in range(W // 128):
                    o = ostp.tile([128, D], dt.float32, tag="ost",
                                  name=f"o_{g}_{tt}")
                    for ec in range(2):
                        psF = psS.tile([128, 512], dt.float32, tag="s",
                                       name=f"psF_{g}_{tt}_{ec}")
                        for fc in range(NCORES):
                            nc.tensor.matmul(
                                psF[:], rxt[:, fc, ts(tt, 128)],
                                w2sb[:, fc, ts(ec, 512)],
                                start=(fc == 0), stop=(fc == NCORES - 1),
                            )
                        # fused bias + PSUM->SBUF copy on DVE (PE is the
                        # critical resource here, DVE has slack)
                        eb.tensor_add(o[:, ts(ec, 512)], psF[:],
                                      b2bc[:, ts(ec, 512)], 512)
                    nc.sync.dma_start(
                        out[base + tt * 128: base + tt * 128 + 128, :], o[:])

            # per-rep carried tiles
            ki_tiles: dict = {}
            state_sb: dict = {}
            state_ps = [None]

            def pa_kq(i: int, m: int):
                """One third of chunk i's x@W1: m=0 -> kT, m=1 -> qT."""
                ps = psA.tile([128, TC], dt.float32, tag="acc",
                              name=f"psa_{i}_{m}")
                for kc in range(D // 128):
                    nc.tensor.matmul(
                        ps[:], w1sb[:, kc, ts(m, FPC)],
                        xsb[:, kc, ts(i, TC)],
                        start=(kc == 0), stop=(kc == D // 128 - 1))
                dst = kTsb if m == 0 else qTsb
                eb.bias_add(dst[:, ts(i, TC)], ps[:], b1sb[:, m:m + 1], TC)

            def pa_v(i: int):
                """Chunk i's V (via PE transpose) and K[s,d] for the state."""
                ps = psA.tile([128, TC], dt.float32, tag="acc",
                              name=f"psa_{i}_2")
                for kc in range(D // 128):
                    nc.tensor.matmul(
                        ps[:], w1sb[:, kc, ts(2, FPC)],
                        xsb[:, kc, ts(i, TC)],
                        start=(kc == 0), stop=(kc == D // 128 - 1))
                vtmp = stage.tile([128, TC], dt.bfloat16, tag="vtmp",
                                  name=f"vtmp_{i}")
                eb.bias_add(vtmp[:], ps[:], b1sb[:, 2:3], TC)
                pv = psT.tile([128, NB, SB], dt.bfloat16, tag="pt",
                              name=f"pv_{i}")
                for u in range(NB):
                    nc.tensor.transpose(pv[:, u, :], vtmp[:, ts(u, SB)],
                                        identsb[:])
                eb.copy(vsb[:, NB * i:NB * i + NB, :], pv[:], NB * SB,
                        b16=True)
                if i < NT - 1:
                    pk = psT.tile([128, NB, SB], dt.bfloat16, tag="pt",
                                  name=f"pk_{i}")
                    for u in range(NB):
                        nc.tensor.transpose(
                            pk[:, u, :],
                            kTsb[:, i * TC + u * SB: i * TC + (u + 1) * SB],
                            identsb[:])
                    kst = ksp.tile([128, NB, SB], dt.bfloat16, tag="ki",
                                   name=f"ki_{i}")
                    eb.copy(kst[:], pk[:], NB * SB, b16=True)
                    ki_tiles[i] = kst

            def attention(i: int, fillers):
                """Chunk i attention. `fillers` is a list of closures issuing
                independent PE work (next chunk's phase_a parts); they are
                woven between the score matmuls and the AV matmuls so the PE
                never idles while DVE/ACT drain the score mask/copies."""
                otp = psO.tile([128, TC], dt.float32, tag="ot",
                               name=f"otp_{i}")
                sss = []

                def score(r):
                    j = NB * i + r
                    cs = SB * r
                    s0p = psS.tile([128, TC], dt.float32, tag="s",
                                   name=f"s0p_{i}_{r}")
                    s1p = psS.tile([128, TC], dt.float32, tag="s",
                                   name=f"s1p_{i}_{r}")
                    nc.tensor.matmul(
                        s0p[:, cs:TC], kTsb[0:64, ts(j, SB)],
                        qTsb[0:64, i * TC + cs:(i + 1) * TC],
                        start=True, stop=True, tile_position=(0, 0))
                    nc.tensor.matmul(
                        s1p[:, cs:TC], kTsb[64:128, ts(j, SB)],
                        qTsb[64:128, i * TC + cs:(i + 1) * TC],
                        start=True, stop=True, tile_position=(64, 0))
                    s0s = sp.tile([128, TC], dt.bfloat16, tag="ss",
                                  name=f"s0s_{i}_{r}")
                    s1s = sp.tile([128, TC], dt.bfloat16, tag="ss",
                                  name=f"s1s_{i}_{r}")
                    me = cs + SB
                    eb.mul_mask(s0s[:, cs:me], s0p[:, cs:me], masksb[:], SB)
                    eb.mul_mask(s1s[:, cs:me], s1p[:, cs:me], masksb[:], SB)
                    if me < TC:
                        eb.copy(s0s[:, me:TC], s0p[:, me:TC], TC - me)
                        eb.copy(s1s[:, me:TC], s1p[:, me:TC], TC - me)
                    sss.append((s0s, s1s))

                def av(r):
                    j = NB * i + r
                    cs = SB * r
                    s0s, s1s = sss[r]
                    st_first = (i == 0 and r == 0)
                    nc.tensor.matmul(
                        otp[0:64, cs:TC], vsb[:, j, 0:64], s0s[:, cs:TC],
                        start=st_first, stop=(r == NB - 1), tile_position=(0, 0))
                    nc.tensor.matmul(
                        otp[64:128, cs:TC], vsb[:, j, 64:128], s1s[:, cs:TC],
                        start=st_first, stop=(r == NB - 1), tile_position=(0, 64))

                score(0)
                if i > 0:
                    st = state_sb[i - 1]
                    nc.tensor.matmul(
                        otp[0:64, :], st[0:64, 0:64], qTsb[0:64, ts(i, TC)],
                        start=True, stop=False, tile_position=(0, 0))
                    nc.tensor.matmul(
                        otp[64:128, :], st[64:128, 64:128],
                        qTsb[64:128, ts(i, TC)],
                        start=True, stop=False, tile_position=(64, 64))
                score(1)
                if fillers:
                    fillers[0]()          # independent PE work
                av(0)
                if len(fillers) > 1:
                    fillers[1]()
                if len(fillers) > 2:
                    fillers[2]()
                av(1)
                fillers = []
                # state update: State += K_i^T V_i (skipped for last chunk).
                # Full 128x128 matmuls: the cross-head quadrants hold garbage
                # that the inter matmuls never read (they use the diagonal
                # 64x64 blocks only).
                if i < NT - 1:
                    kst = ki_tiles[i]
                    for u in range(NB):
                        su_first = (i == 0 and u == 0)
                        su_last = (i == NT - 2 and u == NB - 1)
                        nc.tensor.matmul(
                            state_ps[0][:], kst[:, u, :],
                            vsb[:, NB * i + u, :],
                            start=su_first, stop=su_last,
                            skip_group_check=True)
                    stt = stp.tile([128, 128], dt.bfloat16, tag="st",
                                   name=f"st_{i}")
                    eb.copy(stt[:], state_ps[0][:], 128)
                    state_sb[i] = stt
                for f in fillers[2:]:
                    f()
                att = stage.tile([128, TC], dt.bfloat16, tag="att",
                                 name=f"att_{i}")
                eb.copy(att[:], otp[:], TC)
                att_store(i, att)

            # ---- schedule --------------------------------------------------
            for _rep in range(reps):
                ki_tiles.clear()
                state_sb.clear()
                state_ps[0] = psStat.tile([128, 128], dt.float32, tag="state",
                                          name=f"state_{_rep}")
                rxt0 = None
                pa_kq(0, 0)
                pa_kq(0, 1)
                if _rep == 0:
                    deferred_loads_1()
                pa_v(0)
                if _rep == 0:
                    deferred_loads_2()
                for i in range(NT):
                    if i + 1 < NT:
                        fillers = [lambda i=i: pa_kq(i + 1, 0),
                                   lambda i=i: pa_kq(i + 1, 1),
                                   lambda i=i: pa_v(i + 1)]
                    else:
                        fillers = []
                    attention(i, fillers)
                    if i == GROUPS[0][0][-1]:
                        a2a_launch(0)
                        rxt0 = rx_load(0)
                a2a_launch(1)
                rxt1 = rx_load(1)
                w2_group(0, rxt0)
                w2_group(1, rxt1)

    nc.compile()
    return nc


_PROGRAM_CACHE: list = []


def _get_program() -> bass.Bass:
    if not _PROGRAM_CACHE:
        _PROGRAM_CACHE.append(build_program())
    return _PROGRAM_CACHE[0]


def _make_in_maps(x, W1, b1, W2, b2):
    import ml_dtypes
    bf16 = ml_dtypes.bfloat16
    xTb = np.ascontiguousarray(np.asarray(x, np.float32).T).astype(bf16)
    w2b = np.asarray(W2, np.float32).astype(bf16)
    b2f = np.ascontiguousarray(b2, dtype=np.float32)
    maskt = np.zeros((SB, SB), dtype=np.float32)
    for p in range(SB):
        maskt[p, p:] = 1.0
    identm = np.eye(SB, dtype=np.float32).astype(bf16)

    in_maps = []
    for c in range(NCORES):
        cols = slice(FPC * c, FPC * (c + 1))
        w1c = np.concatenate(
            [W1[:, 0 * D:][:, cols], W1[:, 1 * D:][:, cols], W1[:, 2 * D:][:, cols]],
            axis=1)
        b1c = np.stack(
            [b1[0 * D:][cols], b1[1 * D:][cols], b1[2 * D:][cols]], axis=0)
        in_maps.append({
            "xT": xTb,
            "w1l": np.asarray(w1c, np.float32).astype(bf16),
            "b1l": np.ascontiguousarray(b1c, dtype=np.float32),
            "w2": w2b,
            "b2": b2f,
            "masktri": maskt,
            "ident": identm,
        })
    return in_maps


def kernel(x, W1, b1, W2, b2, _trace=False, **trace_kwargs):
    x = np.asarray(x)
    W1, b1, W2, b2 = (np.asarray(a) for a in (W1, b1, W2, b2))
    nc = _get_program()
    in_maps = _make_in_maps(x, W1, b1, W2, b2)
    res = run_bass_kernel_spmd(
        nc, in_maps, core_ids=list(range(NCORES)), trace=_trace, **trace_kwargs)
    full = np.empty((T, D), dtype=np.float32)
    for c in range(NCORES):
        o = res.results[c]["out"]
        for chunks, W, base in GROUPS:
            L = np.arange(W * c, W * c + W)
            tglob = TC * np.asarray(chunks)[L // TC] + (L % TC)
            full[tglob] = o[base: base + W]
    if _trace:
        kernel.last_results = res
    return full


# revision 34
# speedup vs baseline: 1.3367x; 1.3367x over previous
"""Trainium2 Bass kernel for causal retention multi-head attention.

    kqv = x @ W1 + b1 ; k,q,v = split(kqv)
    out_h = tril_mask(q_h k_h^T) v_h        (per head, no softmax)
    out = concat_heads @ W2 + b2

Sharding: tensor-parallel over heads (2 heads per core). Retention is linear
attention, so inter-chunk contributions collapse into a running 64x64 state
per head:

    O_i = Q_i @ State_{<i} + tril(Q_i K_i^T) V_i
    State_{<i+1} = State_{<i} + K_i^T V_i

Only the block-diagonal 512x512 score tiles are materialized; everything
before the current chunk flows through the state. This cuts attention matmul
work ~5x and PSUM->SBUF score copies ~4x vs. materializing the full causal
staircase.

After attention, an AllToAll exchanges head-feature slices for sequence
slices so each core applies the full W2 to its rows. Group {0..5} launches
after chunk 5 (hidden by remaining compute); the small {6,7} group forms the
tail.
"""
import numpy as np

import concourse.bacc as bacc
import concourse.bass as bass
import concourse.mybir as mybir
import concourse.tile as tile
from concourse.bass_utils import run_bass_kernel_spmd

dt = mybir.dt
ts = bass.ts

T = 4096          # sequence length
D = 1024          # embed dim
NCORES = 8
FPC = D // NCORES  # feature columns per core = 128 (2 heads x 64)
DH = 64           # head dim
TC = 512          # t-chunk
NT = T // TC      # 8 chunks
SB = 128          # s-block within the diagonal chunk

# A2A exchange groups: (chunk list, shard width, out row base)
GROUPS = [([0, 1, 2, 3], 256, 0), ([4, 5, 6, 7], 256, 256)]


class EngineBalancer:
    """Greedy assignment of PSUM->SBUF ops to the less-loaded of DVE/ACT.

    Costs in ns from the TRN2 cost model: DVE 1.042 ns/col (0.521 with the
    2x packed-16-bit mode), ACT 0.833 ns/col, plus memory-access init.
    tensor_tensor ops exist only on DVE.
    """

    def __init__(self, nc):
        self.nc = nc
        self.busy = {"dve": 0.0, "act": 1300.0}  # ACT pays a one-time LUT load

    def _dve_cost(self, cols, psum, b16):
        return cols * (0.521 if b16 else 1.042) + (125.0 if psum else 60.0)

    def _act_cost(self, cols, psum):
        return cols * 0.833 + (143.0 if psum else 185.0)

    def _pick(self, dve_cost, act_cost):
        if self.busy["dve"] + dve_cost <= self.busy["act"] + act_cost:
            self.busy["dve"] += dve_cost
            return "dve"
        self.busy["act"] += act_cost
        return "act"

    def copy(self, out_ap, in_ap, cols, psum=True, b16=False):
        if self._pick(self._dve_cost(cols, psum, b16),
                      self._act_cost(cols, psum)) == "dve":
            self.nc.vector.tensor_copy(out_ap, in_ap)
        else:
            self.nc.scalar.copy(out_ap, in_ap)

    def mul_mask(self, out_ap, in_ap, mask_ap, cols):
        # tensor_tensor only exists on DVE
        self.busy["dve"] += self._dve_cost(cols, True, False)
        self.nc.vector.tensor_mul(out_ap, in_ap, mask_ap)

    def bias_add(self, out_ap, in_ap, bias_ap, cols):
        if self._pick(self._dve_cost(cols, True, False),
                      self._act_cost(cols, True)) == "dve":
            self.nc.vector.tensor_scalar_add(out_ap, in_ap, bias_ap)
        else:
            self.nc.scalar.activation(
                out_ap, in_ap, mybir.ActivationFunctionType.Identity,
                bias=bias_ap, scale=1.0)

    def tensor_add(self, out_ap, a_ap, b_ap, cols):
        self.busy["dve"] += self._dve_cost(cols, True, False)
        self.nc.vector.tensor_add(out_ap, a_ap, b_ap)


def build_program(reps: int = 1, xsplit: int = 8,
                  fake_cc: bool = False) -> bass.Bass:
    nc = bacc.Bacc("TRN2", target_bir_lowering=False, debug=False,
                   num_devices=NCORES)

    xT = nc.dram_tensor("xT", [D, T], dt.bfloat16, kind="ExternalInput")
    w1l = nc.dram_tensor("w1l", [D, 3 * FPC], dt.bfloat16, kind="ExternalInput")
    b1l = nc.dram_tensor("b1l", [3, FPC], dt.float32, kind="ExternalInput")
    w2 = nc.dram_tensor("w2", [D, D], dt.bfloat16, kind="ExternalInput")
    b2 = nc.dram_tensor("b2", [D], dt.float32, kind="ExternalInput")
    masktri = nc.dram_tensor("masktri", [SB, SB], dt.float32,
                             kind="ExternalInput")
    ident = nc.dram_tensor("ident", [SB, SB], dt.bfloat16, kind="ExternalInput")
    out = nc.dram_tensor("out", [TC, D], dt.float32, kind="ExternalOutput")

    rg = [list(range(NCORES))]

    with tile.TileContext(nc) as tc:
        with (
            tc.tile_pool(name="res", bufs=1) as res,          # resident tensors
            tc.tile_pool(name="stage", bufs=3) as stage,      # vtmp / att staging
            tc.tile_pool(name="ksp", bufs=2) as ksp,          # K[s,d] per chunk
            tc.tile_pool(name="sp", bufs=4) as sp,            # masked score tiles
            tc.tile_pool(name="stp", bufs=2) as stp,          # state bf16
            tc.tile_pool(name="ost", bufs=4) as ostp,         # W2 output staging
            tc.tile_pool(name="rx", bufs=2) as rxp,          # post-A2A tiles
            tc.tile_pool(name="psA", bufs=2, space="PSUM") as psA,
            tc.tile_pool(name="psT", bufs=1, space="PSUM") as psT,
            tc.tile_pool(name="psS", bufs=3, space="PSUM") as psS,
            tc.tile_pool(name="psO", bufs=1, space="PSUM") as psO,
            tc.tile_pool(name="psStat", bufs=1, space="PSUM") as psStat,
            tc.tile_pool(name="dram", bufs=1, space="DRAM") as dram,
        ):
            eb = EngineBalancer(nc)
            # ---- resident loads -------------------------------------------
            w1sb = res.tile([128, D // 128, 3 * FPC], dt.bfloat16, tag="w1")
            nc.sync.dma_start(w1sb[:], w1l.ap().rearrange("(a p) c -> p a c", p=128))
            b1sb = res.tile([FPC, 3], dt.float32, tag="b1")
            nc.sync.dma_start(b1sb[:], b1l.ap().rearrange("m p -> p m"))
            identsb = res.tile([SB, SB], dt.bfloat16, tag="ident")
            nc.sync.dma_start(identsb[:], ident.ap())
            masksb = res.tile([SB, SB], dt.float32, tag="masktri")

            xsb = res.tile([128, D // 128, T], dt.bfloat16, tag="x", name="xsb")
            seg = T // xsplit
            for sg in range(xsplit):
                for kc in range(D // 128):
                    nc.sync.dma_start(
                        xsb[:, kc, sg * seg:(sg + 1) * seg],
                        xT[kc * 128:(kc + 1) * 128, sg * seg:(sg + 1) * seg])

            w2sb = res.tile([128, D // 128, D], dt.bfloat16, tag="w2")
            b2row = res.tile([1, D], dt.float32, tag="b2row")
            b2bc = res.tile([128, D], dt.float32, tag="b2bc")

            def deferred_loads_1():
                nc.sync.dma_start(masksb[:], masktri.ap())

            def deferred_loads_2():
                nc.sync.dma_start(w2sb[:],
                                  w2.ap().rearrange("(a p) c -> p a c", p=128))
                nc.sync.dma_start(b2row[:],
                                  b2.ap().rearrange("(o c) -> o c", o=1))
                nc.gpsimd.partition_broadcast(b2bc[:], b2row[:])

            kTsb = res.tile([FPC, T], dt.bfloat16, tag="kT")
            qTsb = res.tile([FPC, T], dt.bfloat16, tag="qT")
            vsb = res.tile([SB, T // SB, FPC], dt.bfloat16, tag="v")

            # A2A staging
            a2a_in, a2a_out = [], []
            for g, (chunks, W, base) in enumerate(GROUPS):
                a2a_in.append(dram.tile([NCORES, FPC, W], dt.bfloat16,
                                        tag=f"ain{g}", name=f"a2a_in{g}"))
                a2a_out.append(dram.tile([NCORES, FPC, W], dt.bfloat16,
                                         tag=f"aout{g}", name=f"a2a_out{g}"))
            chunk_group = {}
            for g, (chunks, W, base) in enumerate(GROUPS):
                for pos, i in enumerate(chunks):
                    chunk_group[i] = (g, pos)

            def a2a_launch(g: int):
                if fake_cc:
                    nc.gpsimd.dma_start(a2a_out[g][:], a2a_in[g][:])
                    return
                nc.gpsimd.collective_compute(
                    "AllToAll", mybir.AluOpType.bypass, replica_groups=rg,
                    ins=[a2a_in[g].opt()], outs=[a2a_out[g].opt()],
                )

            def rx_load(g: int):
                """One rearranged DMA per group. Queue choice matters: a DMA
                that waits on its collective parks in the issuing queue and
                yields to ALL later-issued ready work there, so rx0 sits
                alone on SP (fires the moment collective 0 lands) and rx1
                follows launch(1) in the Pool queue (in-order, no inversion).
                """
                W = GROUPS[g][1]
                r = rxp.tile([FPC, NCORES, W], dt.bfloat16, tag=f"rx{g}",
                             name=f"rx{g}")
                eng = nc.sync if g == 0 else nc.gpsimd
                eng.dma_start(r[:], a2a_out[g].rearrange("f p w -> p f w"))
                return r

            def att_store(i, att):
                g, pos = chunk_group[i]
                W = GROUPS[g][1]
                a = TC * pos
                while a < TC * pos + TC:
                    s = a // W
                    e = min((s + 1) * W, TC * pos + TC)
                    nc.sync.dma_start(
                        a2a_in[g][s][:, a - s * W: e - s * W],
                        att[:, a - TC * pos: e - TC * pos])
                    a = e

            def w2_group(g: int, rxt):
                chunks, W, base = GROUPS[g]
                for tt in range(W // 128):
                    o = ostp.tile([128, D], dt.float32, tag="ost",
                                  name=f"o_{g}_{tt}")
                    for ec in range(2):
                        psF = psS.tile([128, 512], dt.float32, tag="s",
                                       name=f"psF_{g}_{tt}_{ec}")
                        for fc in range(NCORES):
                            nc.tensor.matmul(
                                psF[:], rxt[:, fc, ts(tt, 128)],
                                w2sb[:, fc, ts(ec, 512)],
                                start=(fc == 0), stop=(fc == NCORES - 1),
                            )
                        # fused bias + PSUM->SBUF copy on DVE (PE is the
                        # critical resource here, DVE has slack)
                        eb.tensor_add(o[:, ts(ec, 512)], psF[:],
                                      b2bc[:, ts(ec, 512)], 512)
                    nc.sync.dma_start(
                        out[base + tt * 128: base + tt * 128 + 128, :], o[:])

            # per-rep carried tiles
            ki_tiles: dict = {}
            state_sb: dict = {}
            state_ps = [None]

            def pa_kq(i: int, m: int):
                """One third of chunk i's x@W1: m=0 -> kT, m=1 -> qT."""
                ps = psA.tile([128, TC], dt.float32, tag="acc",
                              name=f"psa_{i}_{m}")
                for kc in range(D // 128):
                    nc.tensor.matmul(
                        ps[:], w1sb[:, kc, ts(m, FPC)],
                        xsb[:, kc, ts(i, TC)],
                        start=(kc == 0), stop=(kc == D // 128 - 1))
                dst = kTsb if m == 0 else qTsb
                eb.bias_add(dst[:, ts(i, TC)], ps[:], b1sb[:, m:m + 1], TC)

            def pa_v(i: int):
                """Chunk i's V (via PE transpose) and K[s,d] for the state."""
                ps = psA.tile([128, TC], dt.float32, tag="acc",
                              name=f"psa_{i}_2")
                for kc in range(D // 128):
                    nc.tensor.matmul(
                        ps[:], w1sb[:, kc, ts(2, FPC)],
                        xsb[:, kc, ts(i, TC)],
                        start=(kc == 0), stop=(kc == D // 128 - 1))
                vtmp = stage.tile([128, TC], dt.bfloat16, tag="vtmp",
                                  name=f"vtmp_{i}")
                eb.bias_add(vtmp[:], ps[:], b1sb[:, 2:3], TC)
                pv = psT.tile([128, 4, SB], dt.bfloat16, tag="pt",
                              name=f"pv_{i}")
                for u in range(4):
                    nc.tensor.transpose(pv[:, u, :], vtmp[:, ts(u, SB)],
                                        identsb[:])
                eb.copy(vsb[:, 4 * i:4 * i + 4, :], pv[:], 512, b16=True)
                if i < NT - 1:
                    pk = psT.tile([128, 4, SB], dt.bfloat16, tag="pt",
                                  name=f"pk_{i}")
                    for u in range(4):
                        nc.tensor.transpose(
                            pk[:, u, :],
                            kTsb[:, i * TC + u * SB: i * TC + (u + 1) * SB],
                            identsb[:])
                    kst = ksp.tile([128, 4, SB], dt.bfloat16, tag="ki",
                                   name=f"ki_{i}")
                    eb.copy(kst[:], pk[:], 512, b16=True)
                    ki_tiles[i] = kst

            def attention(i: int, fillers):
                """Chunk i attention. `fillers` is a list of closures issuing
                independent PE work (next chunk's phase_a parts); they are
                woven between the score matmuls and the AV matmuls so the PE
                never idles while DVE/ACT drain the score mask/copies."""
                otp = psO.tile([128, TC], dt.float32, tag="ot",
                               name=f"otp_{i}")
                sss = []

                def score(r):
                    j = 4 * i + r
                    cs = SB * r
                    s0p = psS.tile([128, TC], dt.float32, tag="s",
                                   name=f"s0p_{i}_{r}")
                    s1p = psS.tile([128, TC], dt.float32, tag="s",
                                   name=f"s1p_{i}_{r}")
                    nc.tensor.matmul(
                        s0p[:, cs:TC], kTsb[0:64, ts(j, SB)],
                        qTsb[0:64, i * TC + cs:(i + 1) * TC],
                        start=True, stop=True, tile_position=(0, 0))
                    nc.tensor.matmul(
                        s1p[:, cs:TC], kTsb[64:128, ts(j, SB)],
                        qTsb[64:128, i * TC + cs:(i + 1) * TC],
                        start=True, stop=True, tile_position=(64, 0))
                    s0s = sp.tile([128, TC], dt.bfloat16, tag="ss",
                                  name=f"s0s_{i}_{r}")
                    s1s = sp.tile([128, TC], dt.bfloat16, tag="ss",
                                  name=f"s1s_{i}_{r}")
                    me = cs + SB
                    eb.mul_mask(s0s[:, cs:me], s0p[:, cs:me], masksb[:], SB)
                    eb.mul_mask(s1s[:, cs:me], s1p[:, cs:me], masksb[:], SB)
                    if me < TC:
                        eb.copy(s0s[:, me:TC], s0p[:, me:TC], TC - me)
                        eb.copy(s1s[:, me:TC], s1p[:, me:TC], TC - me)
                    sss.append((s0s, s1s))

                def av(r):
                    j = 4 * i + r
                    cs = SB * r
                    s0s, s1s = sss[r]
                    st_first = (i == 0 and r == 0)
                    nc.tensor.matmul(
                        otp[0:64, cs:TC], vsb[:, j, 0:64], s0s[:, cs:TC],
                        start=st_first, stop=(r == 3), tile_position=(0, 0))
                    nc.tensor.matmul(
                        otp[64:128, cs:TC], vsb[:, j, 64:128], s1s[:, cs:TC],
                        start=st_first, stop=(r == 3), tile_position=(0, 64))

                score(0)
                if i > 0:
                    st = state_sb[i - 1]
                    nc.tensor.matmul(
                        otp[0:64, :], st[0:64, 0:64], qTsb[0:64, ts(i, TC)],
                        start=True, stop=False, tile_position=(0, 0))
                    nc.tensor.matmul(
                        otp[64:128, :], st[64:128, 64:128],
                        qTsb[64:128, ts(i, TC)],
                        start=True, stop=False, tile_position=(64, 64))
                score(1)
                if fillers:
                    fillers[0]()          # ~1.7us of independent PE work
                av(0)
                score(2)
                av(1)
                score(3)
                if len(fillers) > 1:
                    fillers[1]()
                av(2)
                av(3)
                # state update: State += K_i^T V_i (skipped for last chunk).
                # Full 128x128 matmuls: the cross-head quadrants hold garbage
                # that the inter matmuls never read (they use the diagonal
                # 64x64 blocks only).
                if i < NT - 1:
                    kst = ki_tiles[i]
                    for u in range(4):
                        su_first = (i == 0 and u == 0)
                        su_last = (i == NT - 2 and u == 3)
                        nc.tensor.matmul(
                            state_ps[0][:], kst[:, u, :],
                            vsb[:, 4 * i + u, :],
                            start=su_first, stop=su_last,
                            skip_group_check=True)
                    stt = stp.tile([128, 128], dt.bfloat16, tag="st",
                                   name=f"st_{i}")
                    eb.copy(stt[:], state_ps[0][:], 128)
                    state_sb[i] = stt
                for f in fillers[2:]:
                    f()
                att = stage.tile([128, TC], dt.bfloat16, tag="att",
                                 name=f"att_{i}")
                eb.copy(att[:], otp[:], TC)
                att_store(i, att)

            # ---- schedule --------------------------------------------------
            for _rep in range(reps):
                ki_tiles.clear()
                state_sb.clear()
                state_ps[0] = psStat.tile([128, 128], dt.float32, tag="state",
                                          name=f"state_{_rep}")
                rxt0 = None
                pa_kq(0, 0)
                pa_kq(0, 1)
                if _rep == 0:
                    deferred_loads_1()
                pa_v(0)
                if _rep == 0:
                    deferred_loads_2()
                for i in range(NT):
                    if i + 1 < NT:
                        fillers = [lambda i=i: pa_kq(i + 1, 0),
                                   lambda i=i: pa_kq(i + 1, 1),
                                   lambda i=i: pa_v(i + 1)]
                    else:
                        fillers = []
                    attention(i, fillers)
                    if i == GROUPS[0][0][-1]:
                        a2a_launch(0)
                        rxt0 = rx_load(0)
                a2a_launch(1)
                rxt1 = rx_load(1)
                w2_group(0, rxt0)
                w2_group(1, rxt1)

    nc.compile()
    return nc


_PROGRAM_CACHE: list = []


def _get_program() -> bass.Bass:
    if not _PROGRAM_CACHE:
        _PROGRAM_CACHE.append(build_program())
    return _PROGRAM_CACHE[0]


def _make_in_maps(x, W1, b1, W2, b2):
    import ml_dtypes
    bf16 = ml_dtypes.bfloat16
    xTb = np.ascontiguousarray(np.asarray(x, np.float32).T).astype(bf16)
    w2b = np.asarray(W2, np.float32).astype(bf16)
    b2f = np.ascontiguousarray(b2, dtype=np.float32)
    maskt = np.zeros((SB, SB), dtype=np.float32)
    for p in range(SB):
        maskt[p, p:] = 1.0
    identm = np.eye(SB, dtype=np.float32).astype(bf16)

    in_maps = []
    for c in range(NCORES):
        cols = slice(FPC * c, FPC * (c + 1))
        w1c = np.concatenate(
            [W1[:, 0 * D:][:, cols], W1[:, 1 * D:][:, cols], W1[:, 2 * D:][:, cols]],
            axis=1)
        b1c = np.stack(
            [b1[0 * D:][cols], b1[1 * D:][cols], b1[2 * D:][cols]], axis=0)
        in_maps.append({
            "xT": xTb,
            "w1l": np.asarray(w1c, np.float32).astype(bf16),
            "b1l": np.ascontiguousarray(b1c, dtype=np.float32),
            "w2": w2b,
            "b2": b2f,
            "masktri": maskt,
            "ident": identm,
        })
    return in_maps


def kernel(x, W1, b1, W2, b2, _trace=False, **trace_kwargs):
    x = np.asarray(x)
    W1, b1, W2, b2 = (np.asarray(a) for a in (W1, b1, W2, b2))
    nc = _get_program()
    in_maps = _make_in_maps(x, W1, b1, W2, b2)
    res = run_bass_kernel_spmd(
        nc, in_maps, core_ids=list(range(NCORES)), trace=_trace, **trace_kwargs)
    full = np.empty((T, D), dtype=np.float32)
    for c in range(NCORES):
        o = res.results[c]["out"]
        for chunks, W, base in GROUPS:
            L = np.arange(W * c, W * c + W)
            tglob = TC * np.asarray(chunks)[L // TC] + (L % TC)
            full[tglob] = o[base: base + W]
    if _trace:
        kernel.last_results = res
    return full
